# revision 10
# baseline (speedup 1.0000x reference)
"""Trainium2 Bass kernel for nn_Node_Transformation.

Reference semantics, for row n:
    out[n] = x[n] @ W.T + b            if node_type[n] == item_id
             emb_weight[node_type[n]]  otherwise

Only ~1/8 of rows take the linear path; every other row is one of 7
constant 128-float vectors. The host-side sharding step therefore groups
each core's rows by node_type (selected rows first, then one contiguous
run per other type, each padded to a 128-row tile boundary). The device
kernel then:
  * reads ONLY the selected rows of x (pre-transposed to [256, S], cast
    to bf16), computes lin = x_sel @ W.T via PE-array matmuls, adds the
    bias in fp32 while moving PSUM->SBUF, and writes it to its own
    output tensor;
  * writes each constant run into a per-group output tensor by
    broadcast-source DMAs (stride-0 fan-out of a [128,1,128] replicated
    tile), groups statically spread over the gpsimd/sync/scalar DMA
    queues. Separate output tensors keep the write streams free of
    false write-write dependencies so all queues run concurrently.
The host scatters device rows back to their original positions.

HBM traffic per core: ~4.3 MB read + ~32.3 MB write -> memory-roofline
~103 us at 358 GB/s (vs ~96 MB and ~270 us for the dense formulation).
"""

import os
import numpy as np
import ml_dtypes

import concourse.bass as bass
import concourse.bacc as bacc
import concourse.mybir as mybir
from concourse.tile import TileContext
from concourse.bass_utils import run_bass_kernel_spmd

# ---- problem constants (hardcoded per contest contract) ----
N = 500000
IN_CH = 256
HID = 128
NUM_T = 8
NCORES = 8
P = 128
NSH = N // NCORES          # 62500 rows per core
KT = 4                     # 128-row tiles per PSUM accumulation group
KW = 16                    # tiles per write chunk (2048 rows, 1 MB)

_CACHE = {}


def _ensure_axon_profile_hook():
    """bass_utils' trace path imports antenv.axon_hooks, which this image
    lacks. Register an equivalent module backed by the axon PJRT .so so
    trace=True (or BASS_TRACE=1) works instead of crashing."""
    try:
        import antenv.axon_hooks  # noqa: F401
        return
    except ImportError:
        pass
    import sys
    import types

    hook = None
    try:
        from trn_agent_boot.trn_boot import _ntff_profile_via_ctypes

        hook = _ntff_profile_via_ctypes("/opt/axon/libaxon_pjrt.so")
    except Exception:
        hook = None
    mod = types.ModuleType("antenv.axon_hooks")
    mod.get_axon_ntff_profile_hook = lambda: hook
    mod.set_axon_ntff_profile_hook = lambda h: None
    sys.modules["antenv.axon_hooks"] = mod
    try:
        import antenv

        antenv.axon_hooks = mod
    except ImportError:
        pass


def _build(S: int, consts: tuple) -> bass.Bass:
    """S: selected-row region size (rows, multiple of KT*128).
    consts: tuple of per-group padded row counts (each a multiple of 128),
    one per non-selected node type, each written to its own output."""
    nc = bacc.Bacc("TRN2")
    f32 = mybir.dt.float32
    bf16 = mybir.dt.bfloat16
    ngroups = len(consts)

    xt_d = nc.dram_tensor("xt", [IN_CH, max(S, 1)], bf16, kind="ExternalInput")
    wt_d = nc.dram_tensor("wt", [IN_CH, HID], bf16, kind="ExternalInput")
    # rows 0..ngroups-1: the group constants; row ngroups: the bias b,
    # tiled KT times along the free dim.
    cb_d = nc.dram_tensor("cb", [ngroups + 1, KT * HID], f32,
                          kind="ExternalInput")
    outl_d = nc.dram_tensor("outl", [max(S, 1), HID], f32,
                            kind="ExternalOutput")
    outc_d = [
        nc.dram_tensor(f"outc{t}", [consts[t], HID], f32, kind="ExternalOutput")
        for t in range(ngroups)
    ]

    def chunk(ten, r0, ktiles):
        return ten[r0 : r0 + ktiles * P, :].rearrange("(k p) h -> p k h", p=P)

    with TileContext(nc) as tc:
        with (
            tc.tile_pool(name="singles", bufs=1) as singles,
            tc.tile_pool(name="xp", bufs=4) as xpool,
            tc.tile_pool(name="op", bufs=4) as opool,
            tc.tile_pool(name="ps", bufs=4, space="PSUM") as pspool,
        ):
            ones1 = singles.tile([1, P], f32)
            nc.vector.memset(ones1[:], 1.0)
            wt_s = singles.tile([P, 2, HID], bf16)
            nc.sync.dma_start(
                out=wt_s[:], in_=wt_d[:].rearrange("(two c) h -> c two h", two=2)
            )

            # Replicate each cb row across all 128 partitions via a
            # ones-matmul. Const tiles are a single [128,1,HID] tile each
            # (fanned out at write time with a stride-0 broadcast source);
            # the bias tile is KT wide to match a PSUM accumulation group.
            const_s = []
            for t in range(ngroups):
                stage_t = singles.tile([1, HID], f32)
                nc.sync.dma_start(out=stage_t[:], in_=cb_d[t : t + 1, 0:HID])
                pc = pspool.tile([P, HID], f32, tag="pc")
                nc.tensor.matmul(out=pc[:], lhsT=ones1[:], rhs=stage_t[:],
                                 start=True, stop=True)
                ct = singles.tile([P, 1, HID], f32)
                nc.scalar.copy(ct[:], pc[:].rearrange("p (k h) -> p k h", k=1))
                const_s.append(ct)
            stage_b = singles.tile([1, KT * HID], f32)
            nc.sync.dma_start(out=stage_b[:], in_=cb_d[ngroups : ngroups + 1, :])
            pb = pspool.tile([P, KT * HID], f32, tag="pc")
            nc.tensor.matmul(out=pb[:], lhsT=ones1[:], rhs=stage_b[:],
                             start=True, stop=True)
            bias_rep = singles.tile([P, KT, HID], f32)
            nc.scalar.copy(bias_rep[:], pb[:].rearrange("p (k h) -> p k h", k=KT))

            # Linear region: S rows in super-groups of KW tiles (one write
            # chunk), each made of KT-tile PSUM accumulation groups.
            stiles = S // P
            for g in range(0, stiles, KW):
                w = min(KW, stiles - g)
                c0 = g * P
                xt0 = xpool.tile([P, KW, P], bf16, tag="x0")
                xt1 = xpool.tile([P, KW, P], bf16, tag="x1")
                nc.sync.dma_start(
                    out=xt0[:, 0:w, :],
                    in_=xt_d[0:P, c0 : c0 + w * P].rearrange(
                        "c (k p) -> c k p", k=w),
                )
                nc.sync.dma_start(
                    out=xt1[:, 0:w, :],
                    in_=xt_d[P : 2 * P, c0 : c0 + w * P].rearrange(
                        "c (k p) -> c k p", k=w),
                )
                o_t = opool.tile([P, KW, HID], f32, tag="o")
                for q in range(0, w, KT):
                    ps = pspool.tile([P, KT, HID], f32, tag="ps")
                    for k in range(KT):
                        nc.tensor.matmul(out=ps[:, k, :],
                                         lhsT=xt0[:, q + k, :],
                                         rhs=wt_s[:, 0, :],
                                         start=True, stop=False)
                        nc.tensor.matmul(out=ps[:, k, :],
                                         lhsT=xt1[:, q + k, :],
                                         rhs=wt_s[:, 1, :],
                                         start=False, stop=True)
                    # PSUM -> SBUF move fused with the fp32 bias add.
                    nc.vector.tensor_tensor(out=o_t[:, q : q + KT, :],
                                            in0=ps[:], in1=bias_rep[:],
                                            op=mybir.AluOpType.add)
                nc.scalar.dma_start(out=chunk(outl_d, c0, w), in_=o_t[:, 0:w, :])

            # Constant regions: broadcast-source writes of the replicated
            # tiles; each group owns one output tensor and one DMA queue.
            queues = [nc.gpsimd, nc.sync, nc.scalar]
            for t in range(ngroups):
                q = queues[t % len(queues)]
                tiles = consts[t] // P
                j = 0
                while j < tiles:
                    w = min(KW, tiles - j)
                    q.dma_start(
                        out=chunk(outc_d[t], j * P, w),
                        in_=const_s[t][:, 0:1, :].to_broadcast([P, w, HID]),
                    )
                    j += w
    nc.compile()
    return nc


def _round_up(v, m):
    return (v + m - 1) // m * m


def _prepare(inputs):
    x = np.ascontiguousarray(np.asarray(inputs["x"], dtype=np.float32))
    nt = np.asarray(inputs["node_type"]).astype(np.int64).ravel()
    item = int(np.asarray(inputs["item_id"]))
    emb = np.asarray(inputs["emb_weight"], dtype=np.float32)
    b = np.asarray(inputs["b"], dtype=np.float32)
    W = np.asarray(inputs["W"], dtype=np.float32)
    wt = np.ascontiguousarray(W.T.astype(ml_dtypes.bfloat16))  # [IN_CH, HID]

    const_types = [t for t in range(NUM_T) if t != item]

    sel_idx, grp_idx = [], []
    for c in range(NCORES):
        nt_c = nt[c * NSH : (c + 1) * NSH]
        sel_idx.append(np.flatnonzero(nt_c == item))
        grp_idx.append([np.flatnonzero(nt_c == t) for t in const_types])

    S = _round_up(max(len(s) for s in sel_idx), KT * P)
    consts = tuple(
        _round_up(max(len(grp_idx[c][g]) for c in range(NCORES)), P)
        for g in range(len(const_types))
    )

    rows = (np.concatenate([emb[const_types], b.reshape(1, HID)], axis=0)
            if const_types else b.reshape(1, HID))
    cb = np.ascontiguousarray(np.tile(rows, (1, KT)), dtype=np.float32)

    in_maps = []
    for c in range(NCORES):
        xt = np.zeros((IN_CH, max(S, 1)), ml_dtypes.bfloat16)
        si = sel_idx[c]
        if len(si):
            xt[:, : len(si)] = x[c * NSH + si].T.astype(ml_dtypes.bfloat16)
        in_maps.append({"xt": xt, "wt": wt, "cb": cb})
    return S, consts, sel_idx, grp_idx, in_maps


def _run(inputs, trace=False):
    _ensure_axon_profile_hook()
    S, consts, sel_idx, grp_idx, in_maps = _prepare(inputs)
    key = (S, consts)
    if key not in _CACHE:
        _CACHE[key] = _build(S, consts)
    nc = _CACHE[key]
    res = run_bass_kernel_spmd(nc, in_maps, core_ids=list(range(NCORES)),
                               trace=trace)
    out = np.empty((N, HID), np.float32)
    for c in range(NCORES):
        r = res.results[c]
        out_c = out[c * NSH : (c + 1) * NSH]
        si = sel_idx[c]
        if len(si):
            out_c[si] = r["outl"][: len(si)]
        for g, gi in enumerate(grp_idx[c]):
            if len(gi):
                out_c[gi] = r[f"outc{g}"][: len(gi)]
    return out, res


def kernel(**inputs) -> np.ndarray:
    out, _ = _run(inputs, trace=bool(os.environ.get("KERNEL_TRACE")))
    return out


# revision 15
# speedup vs baseline: 1.0374x; 1.0374x over previous
"""Trainium2 Bass kernel for nn_Node_Transformation.

Reference semantics, for row n:
    out[n] = x[n] @ W.T + b            if node_type[n] == item_id
             emb_weight[node_type[n]]  otherwise

Only ~1/8 of rows take the linear path; every other row is one of 7
constant 128-float vectors. The host-side sharding step therefore groups
each core's rows by node_type (selected rows first, then one contiguous
run per other type, each padded to a 128-row tile boundary). The device
kernel then:
  * reads ONLY the selected rows of x (pre-transposed to [256, S], cast
    to bf16), computes lin = x_sel @ W.T via PE-array matmuls, adds the
    bias in fp32 while moving PSUM->SBUF, and writes it to its own
    output tensor;
  * writes each constant run into a per-group output tensor by
    broadcast-source DMAs (stride-0 fan-out of a [128,1,128] replicated
    tile), groups statically spread over the gpsimd/sync/scalar DMA
    queues. Separate output tensors keep the write streams free of
    false write-write dependencies so all queues run concurrently.
The host scatters device rows back to their original positions.

HBM traffic per core: ~4.3 MB read + ~32.3 MB write -> memory-roofline
~103 us at 358 GB/s (vs ~96 MB and ~270 us for the dense formulation).
"""

import os
import numpy as np
import ml_dtypes

import concourse.bass as bass
import concourse.bacc as bacc
import concourse.mybir as mybir
from concourse.tile import TileContext
from concourse.bass_utils import run_bass_kernel_spmd

# ---- problem constants (hardcoded per contest contract) ----
N = 500000
IN_CH = 256
HID = 128
NUM_T = 8
NCORES = 8
P = 128
NSH = N // NCORES          # 62500 rows per core
KT = 4                     # 128-row tiles per PSUM accumulation group
KW = 16                    # tiles per write chunk (2048 rows, 1 MB)

_CACHE = {}


def _ensure_axon_profile_hook():
    """bass_utils' trace path imports antenv.axon_hooks, which this image
    lacks. Register an equivalent module backed by the axon PJRT .so so
    trace=True (or BASS_TRACE=1) works instead of crashing."""
    try:
        import antenv.axon_hooks  # noqa: F401
        return
    except ImportError:
        pass
    import sys
    import types

    hook = None
    try:
        from trn_agent_boot.trn_boot import _ntff_profile_via_ctypes

        hook = _ntff_profile_via_ctypes("/opt/axon/libaxon_pjrt.so")
    except Exception:
        hook = None
    mod = types.ModuleType("antenv.axon_hooks")
    mod.get_axon_ntff_profile_hook = lambda: hook
    mod.set_axon_ntff_profile_hook = lambda h: None
    sys.modules["antenv.axon_hooks"] = mod
    try:
        import antenv

        antenv.axon_hooks = mod
    except ImportError:
        pass


def _build(S: int, consts: tuple) -> bass.Bass:
    """S: selected-row region size (rows, multiple of KT*128).
    consts: tuple of per-group padded row counts (each a multiple of 128),
    one per non-selected node type, each written to its own output."""
    nc = bacc.Bacc("TRN2")
    f32 = mybir.dt.float32
    bf16 = mybir.dt.bfloat16
    ngroups = len(consts)

    nquad = (ngroups + 3) // 4
    xt_d = nc.dram_tensor("xt", [IN_CH, max(S, 1)], bf16, kind="ExternalInput")
    wt_d = nc.dram_tensor("wt", [IN_CH, HID], bf16, kind="ExternalInput")
    # rows 0..nquad-1: the group constants packed 4 per row; last row:
    # the bias b tiled KT times.
    cb_d = nc.dram_tensor("cb", [nquad + 1, KT * HID], f32,
                          kind="ExternalInput")
    outl_d = nc.dram_tensor("outl", [max(S, 1), HID], f32,
                            kind="ExternalOutput")
    outc_d = [
        nc.dram_tensor(f"outc{t}", [consts[t], HID], f32, kind="ExternalOutput")
        for t in range(ngroups)
    ]

    def chunk(ten, r0, ktiles):
        return ten[r0 : r0 + ktiles * P, :].rearrange("(k p) h -> p k h", p=P)

    with TileContext(nc) as tc:
        with (
            tc.tile_pool(name="singles", bufs=1) as singles,
            tc.tile_pool(name="xp", bufs=4) as xpool,
            tc.tile_pool(name="op", bufs=4) as opool,
            tc.tile_pool(name="ps", bufs=5, space="PSUM") as pspool,
            tc.tile_pool(name="pssetup", bufs=3, space="PSUM") as setup_ps,
        ):
            ones1 = singles.tile([1, P], f32)
            nc.vector.memset(ones1[:], 1.0)
            wt_s = singles.tile([P, 2, HID], bf16)
            nc.sync.dma_start(
                out=wt_s[:], in_=wt_d[:].rearrange("(two c) h -> c two h", two=2)
            )

            # Replicate the cb rows across all 128 partitions via
            # ones-matmuls, 4 packed columns at a time, in a dedicated
            # PSUM pool so the lin pipeline never delays these. Const
            # write sources are [128,1,HID] slices fanned out at write
            # time with a stride-0 broadcast; the bias tile is KT wide
            # to match a PSUM accumulation group.
            quads = []
            for qd in range(nquad + 1):
                stage_t = singles.tile([1, KT * HID], f32)
                nc.sync.dma_start(out=stage_t[:], in_=cb_d[qd : qd + 1, :])
                pc = setup_ps.tile([P, KT * HID], f32, tag="pc")
                nc.tensor.matmul(out=pc[:], lhsT=ones1[:], rhs=stage_t[:],
                                 start=True, stop=True)
                ct = singles.tile([P, KT, HID], f32)
                nc.scalar.copy(ct[:], pc[:].rearrange("p (k h) -> p k h", k=KT))
                quads.append(ct)
            bias_rep = quads[nquad]
            const_s = [quads[t // 4][:, t % 4 : t % 4 + 1, :]
                       for t in range(ngroups)]

            # Linear region: S rows in super-groups of KW tiles (one write
            # chunk), each made of KT-tile PSUM accumulation groups.
            stiles = S // P
            for g in range(0, stiles, KW):
                w = min(KW, stiles - g)
                c0 = g * P
                xt0 = xpool.tile([P, KW, P], bf16, tag="x0")
                xt1 = xpool.tile([P, KW, P], bf16, tag="x1")
                nc.sync.dma_start(
                    out=xt0[:, 0:w, :],
                    in_=xt_d[0:P, c0 : c0 + w * P].rearrange(
                        "c (k p) -> c k p", k=w),
                )
                nc.sync.dma_start(
                    out=xt1[:, 0:w, :],
                    in_=xt_d[P : 2 * P, c0 : c0 + w * P].rearrange(
                        "c (k p) -> c k p", k=w),
                )
                o_t = opool.tile([P, KW, HID], f32, tag="o")
                for q in range(0, w, KT):
                    ps = pspool.tile([P, KT, HID], f32, tag="ps")
                    for k in range(KT):
                        nc.tensor.matmul(out=ps[:, k, :],
                                         lhsT=xt0[:, q + k, :],
                                         rhs=wt_s[:, 0, :],
                                         start=True, stop=False)
                        nc.tensor.matmul(out=ps[:, k, :],
                                         lhsT=xt1[:, q + k, :],
                                         rhs=wt_s[:, 1, :],
                                         start=False, stop=True)
                    # PSUM -> SBUF move fused with the fp32 bias add.
                    nc.vector.tensor_tensor(out=o_t[:, q : q + KT, :],
                                            in0=ps[:], in1=bias_rep[:],
                                            op=mybir.AluOpType.add)
                nc.scalar.dma_start(out=chunk(outl_d, c0, w), in_=o_t[:, 0:w, :])

            # Constant regions: broadcast-source writes of the replicated
            # tiles; each group owns one output tensor and one DMA queue.
            queues = [nc.gpsimd, nc.sync, nc.scalar]
            for t in range(ngroups):
                q = queues[t % len(queues)]
                tiles = consts[t] // P
                j = 0
                while j < tiles:
                    w = min(KW, tiles - j)
                    q.dma_start(
                        out=chunk(outc_d[t], j * P, w),
                        in_=const_s[t].to_broadcast([P, w, HID]),
                    )
                    j += w
    nc.compile()
    return nc


def _round_up(v, m):
    return (v + m - 1) // m * m


def _prepare(inputs):
    x = np.ascontiguousarray(np.asarray(inputs["x"], dtype=np.float32))
    nt = np.asarray(inputs["node_type"]).astype(np.int64).ravel()
    item = int(np.asarray(inputs["item_id"]))
    emb = np.asarray(inputs["emb_weight"], dtype=np.float32)
    b = np.asarray(inputs["b"], dtype=np.float32)
    W = np.asarray(inputs["W"], dtype=np.float32)
    wt = np.ascontiguousarray(W.T.astype(ml_dtypes.bfloat16))  # [IN_CH, HID]

    const_types = [t for t in range(NUM_T) if t != item]

    sel_idx, grp_idx = [], []
    for c in range(NCORES):
        nt_c = nt[c * NSH : (c + 1) * NSH]
        sel_idx.append(np.flatnonzero(nt_c == item))
        grp_idx.append([np.flatnonzero(nt_c == t) for t in const_types])

    S = _round_up(max(len(s) for s in sel_idx), KT * P)
    consts = tuple(
        _round_up(max(len(grp_idx[c][g]) for c in range(NCORES)), P)
        for g in range(len(const_types))
    )

    # Pack the const rows 4 per cb row; final row is the bias tiled KT x.
    nquad = (len(const_types) + 3) // 4
    cb = np.zeros((nquad + 1, KT * HID), np.float32)
    for g, t in enumerate(const_types):
        cb[g // 4, (g % 4) * HID : (g % 4 + 1) * HID] = emb[t]
    cb[nquad] = np.tile(b, KT)

    in_maps = []
    for c in range(NCORES):
        xt = np.zeros((IN_CH, max(S, 1)), ml_dtypes.bfloat16)
        si = sel_idx[c]
        if len(si):
            xt[:, : len(si)] = x[c * NSH + si].T.astype(ml_dtypes.bfloat16)
        in_maps.append({"xt": xt, "wt": wt, "cb": cb})
    return S, consts, sel_idx, grp_idx, in_maps


def _run(inputs, trace=False):
    _ensure_axon_profile_hook()
    S, consts, sel_idx, grp_idx, in_maps = _prepare(inputs)
    key = (S, consts)
    if key not in _CACHE:
        _CACHE[key] = _build(S, consts)
    nc = _CACHE[key]
    res = run_bass_kernel_spmd(nc, in_maps, core_ids=list(range(NCORES)),
                               trace=trace)
    out = np.empty((N, HID), np.float32)
    for c in range(NCORES):
        r = res.results[c]
        out_c = out[c * NSH : (c + 1) * NSH]
        si = sel_idx[c]
        if len(si):
            out_c[si] = r["outl"][: len(si)]
        for g, gi in enumerate(grp_idx[c]):
            if len(gi):
                out_c[gi] = r[f"outc{g}"][: len(gi)]
    return out, res


def kernel(**inputs) -> np.ndarray:
    out, _ = _run(inputs, trace=bool(os.environ.get("KERNEL_TRACE")))
    return out


# revision 16
# speedup vs baseline: 1.0422x; 1.0046x over previous
"""Trainium2 Bass kernel for nn_Node_Transformation.

Reference semantics, for row n:
    out[n] = x[n] @ W.T + b            if node_type[n] == item_id
             emb_weight[node_type[n]]  otherwise

Only ~1/8 of rows take the linear path; every other row is one of 7
constant 128-float vectors. The host-side sharding step therefore groups
each core's rows by node_type (selected rows first, then one contiguous
run per other type, each padded to a 128-row tile boundary). The device
kernel then:
  * reads ONLY the selected rows of x (pre-transposed to [256, S], cast
    to bf16), computes lin = x_sel @ W.T via PE-array matmuls, adds the
    bias in fp32 while moving PSUM->SBUF, and writes it to its own
    output tensor;
  * writes each constant run into a per-group output tensor by
    broadcast-source DMAs (stride-0 fan-out of a [128,1,128] replicated
    tile), groups statically spread over the gpsimd/sync/scalar DMA
    queues. Separate output tensors keep the write streams free of
    false write-write dependencies so all queues run concurrently.
The host scatters device rows back to their original positions.

HBM traffic per core: ~4.3 MB read + ~32.3 MB write -> memory-roofline
~103 us at 358 GB/s (vs ~96 MB and ~270 us for the dense formulation).
"""

import os
import numpy as np
import ml_dtypes

import concourse.bass as bass
import concourse.bacc as bacc
import concourse.mybir as mybir
from concourse.tile import TileContext
from concourse.bass_utils import run_bass_kernel_spmd

# ---- problem constants (hardcoded per contest contract) ----
N = 500000
IN_CH = 256
HID = 128
NUM_T = 8
NCORES = 8
P = 128
NSH = N // NCORES          # 62500 rows per core
KT = 4                     # 128-row tiles per PSUM accumulation group
KW = 16                    # tiles per write chunk (2048 rows, 1 MB)

_CACHE = {}


def _ensure_axon_profile_hook():
    """bass_utils' trace path imports antenv.axon_hooks, which this image
    lacks. Register an equivalent module backed by the axon PJRT .so so
    trace=True (or BASS_TRACE=1) works instead of crashing."""
    try:
        import antenv.axon_hooks  # noqa: F401
        return
    except ImportError:
        pass
    import sys
    import types

    hook = None
    try:
        from trn_agent_boot.trn_boot import _ntff_profile_via_ctypes

        hook = _ntff_profile_via_ctypes("/opt/axon/libaxon_pjrt.so")
    except Exception:
        hook = None
    mod = types.ModuleType("antenv.axon_hooks")
    mod.get_axon_ntff_profile_hook = lambda: hook
    mod.set_axon_ntff_profile_hook = lambda h: None
    sys.modules["antenv.axon_hooks"] = mod
    try:
        import antenv

        antenv.axon_hooks = mod
    except ImportError:
        pass


def _build(S: int, consts: tuple) -> bass.Bass:
    """S: selected-row region size (rows, multiple of KT*128).
    consts: tuple of per-group padded row counts (each a multiple of 128),
    one per non-selected node type, each written to its own output."""
    nc = bacc.Bacc("TRN2")
    f32 = mybir.dt.float32
    bf16 = mybir.dt.bfloat16
    ngroups = len(consts)

    nquad = (ngroups + 3) // 4
    xt_d = nc.dram_tensor("xt", [IN_CH, max(S, 1)], bf16, kind="ExternalInput")
    wt_d = nc.dram_tensor("wt", [IN_CH, HID], bf16, kind="ExternalInput")
    # rows 0..nquad-1: the group constants packed 4 per row; last row:
    # the bias b tiled KT times.
    cb_d = nc.dram_tensor("cb", [nquad + 1, KT * HID], f32,
                          kind="ExternalInput")
    outl_d = nc.dram_tensor("outl", [max(S, 1), HID], f32,
                            kind="ExternalOutput")
    outc_d = [
        nc.dram_tensor(f"outc{t}", [consts[t], HID], f32, kind="ExternalOutput")
        for t in range(ngroups)
    ]

    def chunk(ten, r0, ktiles):
        return ten[r0 : r0 + ktiles * P, :].rearrange("(k p) h -> p k h", p=P)

    with TileContext(nc) as tc:
        with (
            tc.tile_pool(name="singles", bufs=1) as singles,
            tc.tile_pool(name="xp", bufs=4) as xpool,
            tc.tile_pool(name="op", bufs=4) as opool,
            tc.tile_pool(name="ps", bufs=5, space="PSUM") as pspool,
            tc.tile_pool(name="pssetup", bufs=3, space="PSUM") as setup_ps,
        ):
            ones1 = singles.tile([1, P], f32)
            nc.vector.memset(ones1[:], 1.0)
            wt_s = singles.tile([P, 2, HID], bf16)
            nc.sync.dma_start(
                out=wt_s[:], in_=wt_d[:].rearrange("(two c) h -> c two h", two=2)
            )

            # Replicate the cb rows across all 128 partitions via
            # ones-matmuls, 4 packed columns at a time, in a dedicated
            # PSUM pool so the lin pipeline never delays these. Const
            # write sources are [128,1,HID] slices fanned out at write
            # time with a stride-0 broadcast; the bias tile is KT wide
            # to match a PSUM accumulation group.
            quads = []
            for qd in range(nquad + 1):
                stage_t = singles.tile([1, KT * HID], f32)
                nc.sync.dma_start(out=stage_t[:], in_=cb_d[qd : qd + 1, :])
                pc = setup_ps.tile([P, KT * HID], f32, tag="pc")
                nc.tensor.matmul(out=pc[:], lhsT=ones1[:], rhs=stage_t[:],
                                 start=True, stop=True)
                ct = singles.tile([P, KT, HID], f32)
                nc.vector.tensor_copy(ct[:],
                                      pc[:].rearrange("p (k h) -> p k h", k=KT))
                quads.append(ct)
            bias_rep = quads[nquad]
            const_s = [quads[t // 4][:, t % 4 : t % 4 + 1, :]
                       for t in range(ngroups)]

            # Linear region: S rows in super-groups of KW tiles (one write
            # chunk), each made of KT-tile PSUM accumulation groups.
            stiles = S // P
            for g in range(0, stiles, KW):
                w = min(KW, stiles - g)
                c0 = g * P
                xt0 = xpool.tile([P, KW, P], bf16, tag="x0")
                xt1 = xpool.tile([P, KW, P], bf16, tag="x1")
                nc.sync.dma_start(
                    out=xt0[:, 0:w, :],
                    in_=xt_d[0:P, c0 : c0 + w * P].rearrange(
                        "c (k p) -> c k p", k=w),
                )
                nc.sync.dma_start(
                    out=xt1[:, 0:w, :],
                    in_=xt_d[P : 2 * P, c0 : c0 + w * P].rearrange(
                        "c (k p) -> c k p", k=w),
                )
                o_t = opool.tile([P, KW, HID], f32, tag="o")
                for q in range(0, w, KT):
                    ps = pspool.tile([P, KT, HID], f32, tag="ps")
                    for k in range(KT):
                        nc.tensor.matmul(out=ps[:, k, :],
                                         lhsT=xt0[:, q + k, :],
                                         rhs=wt_s[:, 0, :],
                                         start=True, stop=False)
                        nc.tensor.matmul(out=ps[:, k, :],
                                         lhsT=xt1[:, q + k, :],
                                         rhs=wt_s[:, 1, :],
                                         start=False, stop=True)
                    # PSUM -> SBUF move fused with the fp32 bias add.
                    nc.vector.tensor_tensor(out=o_t[:, q : q + KT, :],
                                            in0=ps[:], in1=bias_rep[:],
                                            op=mybir.AluOpType.add)
                nc.scalar.dma_start(out=chunk(outl_d, c0, w), in_=o_t[:, 0:w, :])

            # Constant regions: broadcast-source writes of the replicated
            # tiles; each group owns one output tensor and one DMA queue.
            queues = [nc.gpsimd, nc.sync, nc.scalar]
            for t in range(ngroups):
                q = queues[t % len(queues)]
                tiles = consts[t] // P
                j = 0
                while j < tiles:
                    w = min(KW, tiles - j)
                    q.dma_start(
                        out=chunk(outc_d[t], j * P, w),
                        in_=const_s[t].to_broadcast([P, w, HID]),
                    )
                    j += w
    nc.compile()
    return nc


def _round_up(v, m):
    return (v + m - 1) // m * m


def _prepare(inputs):
    x = np.ascontiguousarray(np.asarray(inputs["x"], dtype=np.float32))
    nt = np.asarray(inputs["node_type"]).astype(np.int64).ravel()
    item = int(np.asarray(inputs["item_id"]))
    emb = np.asarray(inputs["emb_weight"], dtype=np.float32)
    b = np.asarray(inputs["b"], dtype=np.float32)
    W = np.asarray(inputs["W"], dtype=np.float32)
    wt = np.ascontiguousarray(W.T.astype(ml_dtypes.bfloat16))  # [IN_CH, HID]

    const_types = [t for t in range(NUM_T) if t != item]

    sel_idx, grp_idx = [], []
    for c in range(NCORES):
        nt_c = nt[c * NSH : (c + 1) * NSH]
        sel_idx.append(np.flatnonzero(nt_c == item))
        grp_idx.append([np.flatnonzero(nt_c == t) for t in const_types])

    S = _round_up(max(len(s) for s in sel_idx), KT * P)
    consts = tuple(
        _round_up(max(len(grp_idx[c][g]) for c in range(NCORES)), P)
        for g in range(len(const_types))
    )

    # Pack the const rows 4 per cb row; final row is the bias tiled KT x.
    nquad = (len(const_types) + 3) // 4
    cb = np.zeros((nquad + 1, KT * HID), np.float32)
    for g, t in enumerate(const_types):
        cb[g // 4, (g % 4) * HID : (g % 4 + 1) * HID] = emb[t]
    cb[nquad] = np.tile(b, KT)

    in_maps = []
    for c in range(NCORES):
        xt = np.zeros((IN_CH, max(S, 1)), ml_dtypes.bfloat16)
        si = sel_idx[c]
        if len(si):
            xt[:, : len(si)] = x[c * NSH + si].T.astype(ml_dtypes.bfloat16)
        in_maps.append({"xt": xt, "wt": wt, "cb": cb})
    return S, consts, sel_idx, grp_idx, in_maps


def _run(inputs, trace=False):
    _ensure_axon_profile_hook()
    S, consts, sel_idx, grp_idx, in_maps = _prepare(inputs)
    key = (S, consts)
    if key not in _CACHE:
        _CACHE[key] = _build(S, consts)
    nc = _CACHE[key]
    res = run_bass_kernel_spmd(nc, in_maps, core_ids=list(range(NCORES)),
                               trace=trace)
    out = np.empty((N, HID), np.float32)
    for c in range(NCORES):
        r = res.results[c]
        out_c = out[c * NSH : (c + 1) * NSH]
        si = sel_idx[c]
        if len(si):
            out_c[si] = r["outl"][: len(si)]
        for g, gi in enumerate(grp_idx[c]):
            if len(gi):
                out_c[gi] = r[f"outc{g}"][: len(gi)]
    return out, res


def kernel(**inputs) -> np.ndarray:
    out, _ = _run(inputs, trace=bool(os.environ.get("KERNEL_TRACE")))
    return out
